# revision 5
# baseline (speedup 1.0000x reference)
"""Distributed Trainium2 kernel for nn_Aggregator (segment reduce + MLP + BN).

Strategy (8 NeuronCores, SPMD):
  - Host assigns each of the N segments to one core (snake deal by segment
    size so every core gets a near-identical multiset of segment lengths).
    Each core receives ONLY the edges of its own segments, so no large
    cross-core reduction is needed; only BatchNorm statistics (2x128 floats)
    are all-reduced.
  - Per core, segments become "slots". Edges are shipped in two bf16 layouts:
      stream E (edge-major)  [TE, 128, 129]: tiles of 128 edges x (128 feats
          | valid flag). TensorE computes per-window one-hot matmuls
          P^T @ [X | 1 | X^2] accumulating sum / count / sumsq per slot.
      stream T (feat-major)  [128, LT]: each slot padded to K (mult of 4)
          edge columns; VectorE computes per-slot min / max by log2 folding
          with tensor_tensor max/min at bf16 2x rate.
  - Slot-major epilogue (mean, std) uses per-partition scalar ops; TensorE
    transposes mean/std/count to feat-major; MLP h^T = sum_k W_k^T @ stat_k;
    BN stats all-reduced; normalize + ReLU; output h^T [128, S] per core.
  - Host scatters per-core slot outputs back to the [N, 128] result.

Only layout work (permutation, padding, dtype cast) happens on the host;
all arithmetic (sums, extrema, mean/std, embedding lookup, MLP, BN) runs
on device.
"""

import math
import numpy as np
import ml_dtypes

import concourse.bass as bass
import concourse.bacc as bacc
import concourse.tile as tile
import concourse.mybir as mybir
from concourse import bass_utils

BF16 = ml_dtypes.bfloat16
F32 = np.float32

NCORES = 8
D = 128
OUT = 128
WSLOT = 64         # slots per aggregation window (one PSUM accumulation group)
KGRAN = 4          # segment padding granularity in stream T
CL_T = 4096        # stream-T chunk columns (bf16 elems per partition)
SCHUNK = 512       # slots per MLP/BN chunk
EPS_STD = 1e-5
EPS_BN = 1e-5

dt = mybir.dt


# ----------------------------------------------------------------------------
# Host-side planning (layout only)
# ----------------------------------------------------------------------------

class Plan:
    pass


def _round_up(x, m):
    return (x + m - 1) // m * m


def make_plan(index, N):
    """Assign segments to cores, buckets, slots; build static schedules."""
    E = index.shape[0]
    p = Plan()
    p.E, p.N = E, N

    counts = np.bincount(index, minlength=N)
    order = np.argsort(-counts, kind="stable")  # segments, largest first
    pos = np.arange(N)
    r, q = pos // NCORES, pos % NCORES
    snake = np.where(r % 2 == 0, q, NCORES - 1 - q)
    core_of_rank = snake  # core for the i-th largest segment

    # per-core segment lists in descending-size order
    segs_c = [order[core_of_rank == c] for c in range(NCORES)]

    Kof = np.maximum(KGRAN, _round_up(np.maximum(counts, 1), KGRAN))

    # bucket sizes per K per core -> global max
    allK = sorted(set(int(k) for k in np.unique(Kof)))
    S_K = {}
    for K in allK:
        S_K[K] = max(int(np.sum(Kof[segs_c[c]] == K)) for c in range(NCORES))
    # pad total slots to a multiple of WSLOT via the smallest bucket
    S_total = sum(S_K.values())
    pad = (-S_total) % WSLOT
    K0 = allK[0]
    S_K[K0] += pad
    S_total += pad
    p.S_total = S_total
    p.allK = allK
    p.S_K = S_K

    # per-core slot tables
    p.slot_seg = np.full((NCORES, S_total), -1, np.int64)
    p.slot_cnt = np.zeros((NCORES, S_total), np.int64)
    slot_K = np.zeros(S_total, np.int64)
    off = 0
    for K in allK:
        nK = S_K[K]
        slot_K[off:off + nK] = K
        for c in range(NCORES):
            segs = segs_c[c][Kof[segs_c[c]] == K]  # descending count
            p.slot_seg[c, off:off + len(segs)] = segs
            p.slot_cnt[c, off:off + len(segs)] = counts[segs]
        off += nK
    p.slot_K = slot_K
    p.LT = int(np.sum(slot_K))
    slot_tcol = np.zeros(S_total, np.int64)  # start col of slot in stream T
    np.cumsum(slot_K[:-1], out=slot_tcol[1:])
    p.slot_tcol = slot_tcol

    # stream-T chunk schedule: chunks of <= CL_T cols; regions (uniform K runs)
    # that never split a slot.  regions: (chunk_col_off, S, K, slot_base)
    p.t_chunks = []  # list of (global_col_off, ncols, [regions])
    cur_regions, cur_off, cur_cols = [], 0, 0
    s = 0
    while s < S_total:
        K = int(slot_K[s])
        cap = (CL_T - cur_cols) // K
        if cap == 0:
            p.t_chunks.append((cur_off, cur_cols, cur_regions))
            cur_off += cur_cols
            cur_regions, cur_cols = [], 0
            cap = CL_T // K
        run_end = s
        while run_end < S_total and slot_K[run_end] == K and run_end - s < cap:
            run_end += 1
        take = run_end - s
        cur_regions.append((cur_cols, take, K, s))
        cur_cols += take * K
        s = run_end
    if cur_regions:
        p.t_chunks.append((cur_off, cur_cols, cur_regions))

    # windows: slots [w*WSLOT, (w+1)*WSLOT); tiles per window = global max
    p.nW = S_total // WSLOT
    ew = p.slot_cnt.reshape(NCORES, p.nW, WSLOT).sum(axis=2)  # edges per window
    p.T_w = np.maximum(1, -(-ew.max(axis=0) // 128)).astype(np.int64)  # [nW]
    p.TE = int(p.T_w.sum())
    p.tile_w0 = np.zeros(p.nW, np.int64)  # first tile id of window
    np.cumsum(p.T_w[:-1], out=p.tile_w0[1:])

    p.nSC = -(-S_total // SCHUNK)  # MLP/BN chunks
    return p


def make_core_arrays(p, c, x_bf, index):
    """Build per-core input arrays for core c. x_bf: [E,128] bf16 (+1 zero row
    appended at index E)."""
    E = p.E
    # per-segment edge lists
    order_e = np.argsort(index, kind="stable")
    counts = np.bincount(index, minlength=p.N)
    starts = np.zeros(p.N + 1, np.int64)
    np.cumsum(counts, out=starts[1:])

    S = p.S_total
    # ---- stream T ----
    eT = np.full(p.LT, E, np.int64)  # default: zero row
    for s in range(S):
        seg = p.slot_seg[c, s]
        if seg < 0:
            continue
        cnt = int(p.slot_cnt[c, s])
        K = int(p.slot_K[s])
        col = int(p.slot_tcol[s])
        ids = order_e[starts[seg]:starts[seg] + cnt]
        eT[col:col + cnt] = ids
        if cnt < K:
            eT[col + cnt:col + K] = ids[0] if cnt > 0 else E
    xt = np.ascontiguousarray(x_bf[eT].T)  # [128, LT]

    # ---- stream E ----
    TE = p.TE
    eE = np.full(TE * 128, E, np.int64)
    sE = np.full(TE * 128, -1.0, np.float32)
    for w in range(p.nW):
        base = int(p.tile_w0[w]) * 128
        o = base
        for j in range(WSLOT):
            s = w * WSLOT + j
            seg = p.slot_seg[c, s]
            if seg < 0:
                continue
            cnt = int(p.slot_cnt[c, s])
            ids = order_e[starts[seg]:starts[seg] + cnt]
            eE[o:o + cnt] = ids
            sE[o:o + cnt] = j
            o += cnt
    xe = np.empty((TE * 128, 129), BF16)
    xe[:, :128] = x_bf[eE]
    xe[:, 128] = (eE != E).astype(BF16)  # valid flag (count column)
    xe = xe.reshape(TE, 128, 129)
    se = np.ascontiguousarray(sE.reshape(TE, 128).T.astype(BF16))  # [128, TE]

    vrow = (p.slot_seg[c] >= 0).astype(BF16).reshape(1, S)
    return xt, xe, se, vrow


# ----------------------------------------------------------------------------
# Device kernel builder
# ----------------------------------------------------------------------------

def fold_schedule(K):
    """List of fold widths for one slot of K elems: pairs (w, half)."""
    out = []
    w = K
    while w > 1:
        half = (w + 1) // 2
        out.append((w, half))
        w = half
    return out


def build_kernel(p):
    nc = bacc.Bacc("TRN2", target_bir_lowering=False, debug=False,
                   num_devices=NCORES)
    S = p.S_total
    TE, LT, nW = p.TE, p.LT, p.nW

    # --- I/O ---
    xt_d = nc.dram_tensor("xt", [128, LT], dt.bfloat16, kind="ExternalInput")
    xe_d = nc.dram_tensor("xe", [TE, 128, 129], dt.bfloat16, kind="ExternalInput")
    se_d = nc.dram_tensor("se", [128, TE], dt.bfloat16, kind="ExternalInput")
    vrow_d = nc.dram_tensor("vrow", [1, S], dt.bfloat16, kind="ExternalInput")
    w5_d = nc.dram_tensor("w5", [5, 128, 128], dt.bfloat16, kind="ExternalInput")
    demb_d = nc.dram_tensor("demb", [100, 128], dt.bfloat16, kind="ExternalInput")
    gamma_d = nc.dram_tensor("gamma", [128, 1], dt.float32, kind="ExternalInput")
    beta_d = nc.dram_tensor("beta", [128, 1], dt.float32, kind="ExternalInput")
    iota64_d = nc.dram_tensor("iota64", [128, 64], dt.bfloat16, kind="ExternalInput")
    iotac_d = nc.dram_tensor("iotac", [128, 1], dt.float32, kind="ExternalInput")
    ident64_d = nc.dram_tensor("ident64", [64, 64], dt.bfloat16, kind="ExternalInput")
    ones1_d = nc.dram_tensor("ones1", [1, 128], dt.bfloat16, kind="ExternalInput")
    hout_d = nc.dram_tensor("hout", [128, S], dt.float32, kind="ExternalOutput")

    TWmax = int(p.T_w.max())

    with tile.TileContext(nc) as tc:
        import contextlib
        with contextlib.ExitStack() as ctx:
            cpool = ctx.enter_context(tc.tile_pool(name="const", bufs=1))
            tpool = ctx.enter_context(tc.tile_pool(name="tchunk", bufs=2))
            fpool = ctx.enter_context(tc.tile_pool(name="ftmp", bufs=2))
            epool = ctx.enter_context(tc.tile_pool(name="echunk", bufs=2))
            ppool = ctx.enter_context(tc.tile_pool(name="pchunk", bufs=2))
            stpool = ctx.enter_context(tc.tile_pool(name="stats", bufs=1))
            wpool = ctx.enter_context(tc.tile_pool(name="wtmp", bufs=3))
            mpool = ctx.enter_context(tc.tile_pool(name="mlp", bufs=2))
            hpool = ctx.enter_context(tc.tile_pool(name="hstage", bufs=3))
            psw = ctx.enter_context(tc.tile_pool(name="psw", bufs=2, space="PSUM"))
            pst = ctx.enter_context(tc.tile_pool(name="pst", bufs=2, space="PSUM"))
            psb = ctx.enter_context(tc.tile_pool(name="psb", bufs=1, space="PSUM"))
            psh = ctx.enter_context(tc.tile_pool(name="psh", bufs=2, space="PSUM"))
            dram = ctx.enter_context(tc.tile_pool(name="dram", bufs=1, space="DRAM"))

            # --- constants ---
            iota64 = cpool.tile([128, 64], dt.bfloat16, tag="iota64")
            nc.sync.dma_start(iota64[:], iota64_d.ap())
            iotac = cpool.tile([128, 1], dt.float32, tag="iotac")
            nc.sync.dma_start(iotac[:], iotac_d.ap())
            ident64 = cpool.tile([64, 64], dt.bfloat16, tag="ident64")
            nc.sync.dma_start(ident64[:], ident64_d.ap())
            ones1 = cpool.tile([1, 128], dt.bfloat16, tag="ones1")
            nc.sync.dma_start(ones1[:], ones1_d.ap())
            demb = cpool.tile([100, 128], dt.bfloat16, tag="demb")
            nc.sync.dma_start(demb[:], demb_d.ap())
            w5 = cpool.tile([128, 5 * 128], dt.bfloat16, tag="w5")
            nc.sync.dma_start(
                w5[:].rearrange("p (k f) -> p k f", k=5),
                w5_d.ap().rearrange("k p f -> p k f"))
            gamma = cpool.tile([128, 1], dt.float32, tag="gamma")
            nc.sync.dma_start(gamma[:], gamma_d.ap())
            beta = cpool.tile([128, 1], dt.float32, tag="beta")
            nc.sync.dma_start(beta[:], beta_d.ap())
            vrow = cpool.tile([1, S], dt.bfloat16, tag="vrow")
            nc.sync.dma_start(vrow[:], vrow_d.ap())

            # --- persistent stats (feat-major) ---
            mnT = stpool.tile([128, S], dt.bfloat16, tag="mnT")
            mxT = stpool.tile([128, S], dt.bfloat16, tag="mxT")
            meanT = stpool.tile([128, S], dt.bfloat16, tag="meanT")
            stdT = stpool.tile([128, S], dt.bfloat16, tag="stdT")
            hm = stpool.tile([128, S], dt.bfloat16, tag="hm")
            st_all = stpool.tile([64, nW * 258], dt.bfloat16, tag="st")
            cnt_row = stpool.tile([1, S], dt.bfloat16, tag="cntrow")

            st3 = st_all[:].rearrange("p (w c) -> p w c", c=258)

            # ================= stream T: min / max folds =================
            for (gcol, ncols, regions) in p.t_chunks:
                tch = tpool.tile([128, CL_T], dt.bfloat16, tag="tch")
                nc.sync.dma_start(tch[:, :ncols], xt_d.ap()[:, gcol:gcol + ncols])
                for (roff, rS, K, sbase) in regions:
                    src = tch[:, roff:roff + rS * K].rearrange(
                        "p (s k) -> p s k", k=K)
                    for op, dest, tg in ((mybir.AluOpType.min, mnT, "fmn"),
                                         (mybir.AluOpType.max, mxT, "fmx")):
                        sched = fold_schedule(K)
                        if K == 1:
                            nc.vector.tensor_copy(
                                out=dest[:, sbase:sbase + rS], in_=src[:, :, 0])
                            continue
                        halfK = sched[0][1]
                        tmp = fpool.tile([128, CL_T // 2], dt.bfloat16, tag=tg)
                        t3 = tmp[:, :rS * halfK].rearrange(
                            "p (s k) -> p s k", k=halfK)
                        for li, (w, half) in enumerate(sched):
                            last = li == len(sched) - 1
                            if li == 0:
                                i0 = src[:, :, 0:w - half]
                                i1 = src[:, :, half:w]
                            else:
                                i0 = t3[:, :, 0:w - half]
                                i1 = t3[:, :, half:w]
                            if last:
                                o = dest[:, sbase:sbase + rS]
                            else:
                                o = t3[:, :, 0:w - half]
                            nc.vector.tensor_tensor(out=o, in0=i0, in1=i1, op=op)

            # ================= stream E: windows =================
            for w in range(nW):
                t0 = int(p.tile_w0[w])
                Tw = int(p.T_w[w])
                ech = epool.tile([128, TWmax * 258], dt.bfloat16, tag="ech")
                e3 = ech[:].rearrange("p (t c) -> p t c", c=258)
                nc.sync.dma_start(
                    e3[:, 0:Tw, 0:129],
                    xe_d.ap()[t0:t0 + Tw].rearrange("t e f -> e t f"))
                sch = ppool.tile([128, TWmax], dt.bfloat16, tag="sch")
                nc.sync.dma_start(sch[:, 0:Tw], se_d.ap()[:, t0:t0 + Tw])
                # X^2
                nc.scalar.activation(
                    out=e3[:, 0:Tw, 129:257], in_=e3[:, 0:Tw, 0:128],
                    func=mybir.ActivationFunctionType.Square)
                # one-hot
                pch = ppool.tile([128, TWmax * 64], dt.bfloat16, tag="pch")
                for t in range(Tw):
                    nc.vector.tensor_tensor(
                        out=pch[:, t * 64:(t + 1) * 64],
                        in0=sch[:, t:t + 1].to_broadcast([128, 64]),
                        in1=iota64[:], op=mybir.AluOpType.is_equal)
                # matmuls: psum [64, 257] = P^T @ [X | 1 | X^2]
                ps = psw.tile([64, 258], dt.float32, tag="psw")
                for t in range(Tw):
                    nc.tensor.matmul(
                        out=ps[:, 0:257],
                        lhsT=pch[:, t * 64:(t + 1) * 64],
                        rhs=ech[:, t * 258:t * 258 + 257],
                        start=(t == 0), stop=(t == Tw - 1))
                # evacuate to slot-major staging (bf16)
                nc.scalar.copy(out=st3[:, w, 0:257], in_=ps[:, 0:257])

                # ---- slot-major epilogue for this window ----
                cntf = wpool.tile([64, 1], dt.float32, tag="cntf")
                nc.vector.tensor_copy(out=cntf[:], in_=st3[:, w, 128:129])
                nc.vector.tensor_scalar_max(out=cntf[:], in0=cntf[:], scalar1=1.0)
                rc = wpool.tile([64, 1], dt.float32, tag="rc")
                nc.vector.reciprocal(out=rc[:], in_=cntf[:])
                # mean, mean_sq (in place)
                nc.vector.tensor_scalar(
                    out=st3[:, w, 0:128], in0=st3[:, w, 0:128],
                    scalar1=rc[:], scalar2=None, op0=mybir.AluOpType.mult)
                nc.vector.tensor_scalar(
                    out=st3[:, w, 129:257], in0=st3[:, w, 129:257],
                    scalar1=rc[:], scalar2=None, op0=mybir.AluOpType.mult)
                # var = relu(mean_sq - mean^2); std = sqrt(var + eps)
                vv = wpool.tile([64, 128], dt.float32, tag="vv")
                nc.vector.tensor_tensor(
                    out=vv[:], in0=st3[:, w, 0:128], in1=st3[:, w, 0:128],
                    op=mybir.AluOpType.mult)
                nc.vector.tensor_tensor(
                    out=vv[:], in0=st3[:, w, 129:257], in1=vv[:],
                    op=mybir.AluOpType.subtract)
                nc.vector.tensor_scalar(
                    out=vv[:], in0=vv[:], scalar1=0.0, scalar2=EPS_STD,
                    op0=mybir.AluOpType.max, op1=mybir.AluOpType.add)
                nc.scalar.activation(
                    out=st3[:, w, 129:257], in_=vv[:],
                    func=mybir.ActivationFunctionType.Sqrt)

                # ---- transposes to feat-major ----
                pt = pst.tile([128, 64], dt.bfloat16, tag="pt")
                nc.tensor.transpose(out=pt[:], in_=st3[:, w, 0:128],
                                    identity=ident64[:])
                nc.vector.tensor_copy(out=meanT[:, w * 64:(w + 1) * 64], in_=pt[:])
                pt2 = pst.tile([128, 64], dt.bfloat16, tag="pt")
                nc.tensor.transpose(out=pt2[:], in_=st3[:, w, 129:257],
                                    identity=ident64[:])
                nc.vector.tensor_copy(out=stdT[:, w * 64:(w + 1) * 64], in_=pt2[:])
                ptc = pst.tile([1, 64], dt.bfloat16, tag="pt")
                nc.tensor.transpose(out=ptc[:], in_=st3[:, w, 128:129],
                                    identity=ident64[:])
                nc.vector.tensor_copy(out=cnt_row[:, w * 64:(w + 1) * 64], in_=ptc[:])

            # ================= MLP / BN per slot chunk =================
            sq_parts = stpool.tile([128, p.nSC], dt.float32, tag="sqp")
            for ci in range(p.nSC):
                o0 = ci * SCHUNK
                cw = min(SCHUNK, S - o0)
                # validity broadcast [128, cw]
                pv = psb.tile([128, SCHUNK], dt.float32, tag="pmisc")
                nc.tensor.matmul(out=pv[:, 0:cw], lhsT=ones1[:],
                                 rhs=vrow[:, o0:o0 + cw], start=True, stop=True)
                va = mpool.tile([128, SCHUNK], dt.bfloat16, tag="va")
                nc.scalar.copy(out=va[:, 0:cw], in_=pv[:, 0:cw])
                # deg -> one-hot -> embedding
                dgr = mpool.tile([1, SCHUNK], dt.bfloat16, tag="dgr")
                nc.vector.tensor_scalar_min(
                    out=dgr[:, 0:cw], in0=cnt_row[:, o0:o0 + cw], scalar1=99.0)
                pd = psb.tile([128, SCHUNK], dt.float32, tag="pmisc")
                nc.tensor.matmul(out=pd[0:100, 0:cw], lhsT=ones1[:, 0:100],
                                 rhs=dgr[:, 0:cw], start=True, stop=True)
                d1 = mpool.tile([100, SCHUNK], dt.bfloat16, tag="d1")
                nc.vector.tensor_scalar(
                    out=d1[:, 0:cw], in0=pd[0:100, 0:cw],
                    scalar1=iotac[0:100], scalar2=None,
                    op0=mybir.AluOpType.is_equal)
                pe = psb.tile([128, SCHUNK], dt.float32, tag="pmisc")
                nc.tensor.matmul(out=pe[:, 0:cw], lhsT=demb[:],
                                 rhs=d1[:, 0:cw], start=True, stop=True)
                emb = mpool.tile([128, SCHUNK], dt.bfloat16, tag="emb")
                nc.scalar.copy(out=emb[:, 0:cw], in_=pe[:, 0:cw])
                # h^T = sum_k W_k^T @ stat_k
                ph = psh.tile([128, SCHUNK], dt.float32, tag="ph")
                stats = (meanT, mnT, mxT, stdT)
                for k in range(5):
                    rhs = (stats[k][:, o0:o0 + cw] if k < 4 else emb[:, 0:cw])
                    nc.tensor.matmul(out=ph[:, 0:cw],
                                     lhsT=w5[:, k * 128:(k + 1) * 128],
                                     rhs=rhs, start=(k == 0), stop=(k == 4))
                # mask dummies, stash h
                nc.vector.tensor_tensor(out=hm[:, o0:o0 + cw], in0=ph[:, 0:cw],
                                        in1=va[:, 0:cw], op=mybir.AluOpType.mult)
                # sumsq partial
                hsq = hpool.tile([128, SCHUNK], dt.bfloat16, tag="hsq")
                nc.scalar.activation(
                    out=hsq[:, 0:cw], in_=hm[:, o0:o0 + cw],
                    func=mybir.ActivationFunctionType.Square,
                    accum_out=sq_parts[:, ci:ci + 1])

            # ---- BN stats + AllReduce ----
            bn_in = stpool.tile([128, 2], dt.float32, tag="bnin")
            nc.vector.tensor_reduce(
                out=bn_in[:, 0:1], in_=hm[:], axis=mybir.AxisListType.X,
                op=mybir.AluOpType.add)
            nc.vector.tensor_reduce(
                out=bn_in[:, 1:2], in_=sq_parts[:], axis=mybir.AxisListType.X,
                op=mybir.AluOpType.add)
            bounce_i = dram.tile([128, 2], dt.float32)
            bounce_o = dram.tile([128, 2], dt.float32)
            nc.gpsimd.dma_start(bounce_i[:], bn_in[:])
            nc.gpsimd.collective_compute(
                "AllReduce", mybir.AluOpType.add,
                replica_groups=[list(range(NCORES))],
                ins=[bounce_i.opt()], outs=[bounce_o.opt()])
            bn_out = stpool.tile([128, 2], dt.float32, tag="bnout")
            nc.gpsimd.dma_start(bn_out[:], bounce_o[:])

            inv_n = 1.0 / float(p.N)
            mu = wpool.tile([128, 1], dt.float32, tag="mu")
            nc.vector.tensor_scalar(out=mu[:], in0=bn_out[:, 0:1],
                                    scalar1=inv_n, scalar2=None,
                                    op0=mybir.AluOpType.mult)
            ex2 = wpool.tile([128, 1], dt.float32, tag="ex2")
            nc.vector.tensor_scalar(out=ex2[:], in0=bn_out[:, 1:2],
                                    scalar1=inv_n, scalar2=None,
                                    op0=mybir.AluOpType.mult)
            var = wpool.tile([128, 1], dt.float32, tag="var")
            nc.vector.tensor_tensor(out=var[:], in0=mu[:], in1=mu[:],
                                    op=mybir.AluOpType.mult)
            nc.vector.tensor_tensor(out=var[:], in0=ex2[:], in1=var[:],
                                    op=mybir.AluOpType.subtract)
            nc.vector.tensor_scalar(out=var[:], in0=var[:], scalar1=EPS_BN,
                                    scalar2=None, op0=mybir.AluOpType.add)
            sdv = wpool.tile([128, 1], dt.float32, tag="sdv")
            nc.scalar.activation(out=sdv[:], in_=var[:],
                                 func=mybir.ActivationFunctionType.Sqrt)
            istd = wpool.tile([128, 1], dt.float32, tag="istd")
            nc.vector.reciprocal(out=istd[:], in_=sdv[:])
            scl = wpool.tile([128, 1], dt.float32, tag="scl")
            nc.vector.tensor_tensor(out=scl[:], in0=gamma[:], in1=istd[:],
                                    op=mybir.AluOpType.mult)
            shf = wpool.tile([128, 1], dt.float32, tag="shf")
            nc.vector.tensor_tensor(out=shf[:], in0=mu[:], in1=scl[:],
                                    op=mybir.AluOpType.mult)
            nc.vector.tensor_tensor(out=shf[:], in0=beta[:], in1=shf[:],
                                    op=mybir.AluOpType.subtract)

            # ---- normalize + relu + out ----
            for ci in range(p.nSC):
                o0 = ci * SCHUNK
                cw = min(SCHUNK, S - o0)
                hs = hpool.tile([128, SCHUNK], dt.float32, tag="hs")
                nc.vector.tensor_scalar(
                    out=hs[:, 0:cw], in0=hm[:, o0:o0 + cw],
                    scalar1=scl[:], scalar2=shf[:],
                    op0=mybir.AluOpType.mult, op1=mybir.AluOpType.add)
                nc.vector.tensor_scalar_max(out=hs[:, 0:cw], in0=hs[:, 0:cw],
                                            scalar1=0.0)
                nc.sync.dma_start(hout_d.ap()[:, o0:o0 + cw], hs[:, 0:cw])

    nc.compile()
    return nc


# ----------------------------------------------------------------------------
# Top-level
# ----------------------------------------------------------------------------

def prepare(inputs, index, deg_emb, W, gamma, beta, dim_size):
    N = int(dim_size)
    E = index.shape[0]
    index = np.asarray(index)
    p = make_plan(index, N)

    x_bf = np.empty((E + 1, 128), BF16)
    x_bf[:E] = np.asarray(inputs).astype(BF16)
    x_bf[E] = 0

    in_maps = []
    for c in range(NCORES):
        xt, xe, se, vrow = make_core_arrays(p, c, x_bf, index)
        m = {
            "xt": xt, "xe": xe, "se": se, "vrow": vrow,
            "w5": np.ascontiguousarray(
                np.asarray(W).astype(BF16).reshape(5, 128, 128)),
            "demb": np.asarray(deg_emb).astype(BF16),
            "gamma": np.asarray(gamma).astype(F32).reshape(128, 1),
            "beta": np.asarray(beta).astype(F32).reshape(128, 1),
            "iota64": np.broadcast_to(
                np.arange(64, dtype=BF16), (128, 64)).copy(),
            "iotac": np.arange(128, dtype=F32).reshape(128, 1),
            "ident64": np.eye(64, dtype=BF16),
            "ones1": np.ones((1, 128), BF16),
        }
        in_maps.append(m)

    nc = build_kernel(p)

    def assemble(results):
        out = np.zeros((N, 128), F32)
        for c in range(NCORES):
            hT = results[c]["hout"]  # [128, S]
            segs = p.slot_seg[c]
            mask = segs >= 0
            out[segs[mask]] = hT.T[mask]
        return out

    return nc, in_maps, assemble


def kernel(inputs, index, deg_emb, W, gamma, beta, dim_size):
    nc, in_maps, assemble = prepare(inputs, index, deg_emb, W, gamma, beta,
                                    dim_size)
    res = bass_utils.run_bass_kernel_spmd(
        nc, in_maps, core_ids=list(range(NCORES)))
    return assemble(res.results)
